# revision 17
# baseline (speedup 1.0000x reference)
"""GAT (2-layer) Trainium2 Bass kernel, 8-core SPMD.

Strategy (v2):
- Host: add self-loops, compute per-edge pre-activation attention logits
  (0.4% of FLOPs), shard edges by dst node-range across 8 cores. Per core,
  order edges as: for src-group g (32k-row gather-table slices, int16 idx):
  for dst-block b (128 nodes): edges(b,g), each (b,g) padded to 128-edge
  tiles; tiles padded to fixed 8192-edge gather chunks. All cores padded to
  the identical compile-time tile structure.
- Device (per core, identical SPMD program):
    phase 1: h = x @ W (bf16) for ALL nodes (replicated) -> HBM table
    phase 2: per chunk: dma_gather 256B rows of h[src]; DVE: build one-hot
      Sw[e, dstlocal] (bf16), ex = exp(leaky_relu(al)) (ACT), rhs =
      [ex_h * h_h | ex]; per 128-edge tile: PE matmul psum[b] += Sw^T @ rhs
      accumulating both weighted features and softmax denominators in PSUM;
      per (b,g) run, add psum into an SBUF accumulator.
    phase 3: out[d] = num[d]/den[d] + bias (+ ELU for layer 1)
- Two launches (layer 1, layer 2); host resharding between them.
"""

import os
import numpy as np
from contextlib import ExitStack

import concourse.bass as bass
import concourse.tile as tile
from concourse import bacc, mybir, bass_utils

F32 = mybir.dt.float32
BF16 = mybir.dt.bfloat16
I16 = mybir.dt.int16
AF = mybir.ActivationFunctionType
ALU = mybir.AluOpType

DBG_SKIP = set(os.environ.get("GAT_DBG_SKIP", "").split(","))

N_CORES = 8
P = 128
CHUNK = int(os.environ.get("GAT_CHUNK", "8192"))  # edges per gather chunk
SLOTS = CHUNK // P
IDXF = CHUNK // 16
SRC_CHUNK = 32768     # rows per gather-table slice (int16 index limit)

# problem constants
N = 100000
E = 1600000
IN_DIM = 128
HID = 64
OUT_DIM = 64
H1, H2 = 2, 1

LAST_EXEC_NS = None


def _ceil_to(x, m):
    return (x + m - 1) // m * m


def build_layer_program(cfg):
    """Build + compile the per-layer SPMD program.

    cfg keys:
      n_nodes_pad : gather-table rows (mult of 128)
      hc          : projection width (n_heads*head_dim), <=128
      tw          : table row width in bf16 elems (tw*2 % 256 == 0)
      n_heads, head_dim
      n_blocks    : dst blocks per core (ceil(shard/128))
      out_rows    : output rows (mult of 128, >= shard)
      chunk_tiles : list of chunks; each chunk is a list of SLOTS
                    (block_idx, start, stop) tile descriptors or None (dummy)
      chunk_group : list, src-group per chunk
      apply_elu   : bool
    """
    n_nodes_pad = cfg["n_nodes_pad"]
    hc = cfg["hc"]
    tw = cfg["tw"]
    n_heads = cfg["n_heads"]
    hd = cfg["head_dim"]
    n_blocks = cfg["n_blocks"]
    out_rows = cfg["out_rows"]
    chunk_tiles = cfg["chunk_tiles"]
    chunk_group = cfg["chunk_group"]
    apply_elu = cfg["apply_elu"]
    n_chunks = len(chunk_tiles)
    rw = hc + n_heads              # matmul rhs width
    aw = rw                        # sbuf accum width
    assert hc == n_heads * hd
    assert (tw * 2) % 256 == 0 and tw >= hc
    assert n_nodes_pad % P == 0 and out_rows % P == 0

    nc = bacc.Bacc("TRN2", target_bir_lowering=False, debug=False,
                   num_devices=N_CORES)

    xT = nc.dram_tensor("xT", [P, n_nodes_pad], BF16, kind="ExternalInput")
    W = nc.dram_tensor("W", [P, hc], BF16, kind="ExternalInput")
    biasrep = nc.dram_tensor("biasrep", [P, hc], F32, kind="ExternalInput")
    # per-edge data, gather-wrap layout
    al = nc.dram_tensor("al", [P, n_chunks * SLOTS * n_heads], F32,
                        kind="ExternalInput")
    dstloc = nc.dram_tensor("dstloc", [P, n_chunks * SLOTS], BF16,
                            kind="ExternalInput")
    gidx = nc.dram_tensor("gidx", [P, n_chunks * IDXF], I16,
                          kind="ExternalInput")
    iotaT = nc.dram_tensor("iotaT", [P, P], BF16, kind="ExternalInput")
    htab = nc.dram_tensor("htab", [n_nodes_pad, tw], BF16, kind="Internal")
    out = nc.dram_tensor("out", [out_rows, hc], F32, kind="ExternalOutput")

    with ExitStack() as ctx:
        tc = ctx.enter_context(tile.TileContext(nc))
        cpool = ctx.enter_context(tc.tile_pool(name="const", bufs=1))
        W_sb = cpool.tile([P, hc], BF16)
        nc.sync.dma_start(W_sb[:], W.ap())
        bias_sb = cpool.tile([P, hc], F32)
        nc.sync.dma_start(bias_sb[:], biasrep.ap())
        iota_sb = cpool.tile([P, SLOTS, P], BF16)
        for s in range(SLOTS):
            nc.sync.dma_start(iota_sb[:, s, :], iotaT.ap())
        acc_sb = cpool.tile([P, n_blocks, aw], F32)
        nc.vector.memset(acc_sb[:], 0.0)

        # phase 1: projection -> gather table (bf16)
        xpool = ctx.enter_context(tc.tile_pool(name="xp", bufs=4))
        hpool = ctx.enter_context(tc.tile_pool(name="hp", bufs=4))
        pspool = ctx.enter_context(tc.tile_pool(name="ps", bufs=4,
                                                space="PSUM"))
        BK = 8
        nb_total = n_nodes_pad // P
        for b0 in range(0, nb_total, BK):
            k = min(BK, nb_total - b0)
            xt = xpool.tile([P, BK * P], BF16)
            nc.sync.dma_start(xt[:, 0:k * P], xT.ap()[:, b0 * P:(b0 + k) * P])
            hs = hpool.tile([P, BK, tw], BF16)
            if tw > hc:
                nc.vector.memset(hs[:, :, hc:tw], 0.0)
            for i in range(k):
                ps = pspool.tile([P, hc], F32)
                nc.tensor.matmul(ps[:], xt[:, i * P:(i + 1) * P], W_sb[:],
                                 start=True, stop=True)
                nc.scalar.activation(hs[:, i, 0:hc], ps[:], AF.Copy)
            nc.sync.dma_start(
                htab.ap()[b0 * P:(b0 + k) * P, :].rearrange(
                    "(k p) t -> p k t", p=P),
                hs[:, 0:k, :])

        # phase 2: edges
        ipool = ctx.enter_context(tc.tile_pool(name="ip", bufs=3))
        apool = ctx.enter_context(tc.tile_pool(name="ap", bufs=3))
        gpool = ctx.enter_context(tc.tile_pool(name="gp", bufs=2))
        rpool = ctx.enter_context(tc.tile_pool(name="rp", bufs=2))
        spool = ctx.enter_context(tc.tile_pool(name="sp", bufs=2))
        epool = ctx.enter_context(tc.tile_pool(name="ep", bufs=3))
        mpool = ctx.enter_context(tc.tile_pool(name="mp", bufs=4,
                                               space="PSUM"))
        cur_ps = None   # open accumulation run: (psum_tile, block)

        def close_run():
            nonlocal cur_ps
            if cur_ps is not None:
                pst, blk = cur_ps
                nc.vector.tensor_add(acc_sb[:, blk, :], acc_sb[:, blk, :],
                                     pst[:])
                cur_ps = None

        for ck in range(n_chunks):
            q = chunk_group[ck]
            r0 = q * SRC_CHUNK
            r1 = min(r0 + SRC_CHUNK, n_nodes_pad)
            gi = ipool.tile([P, IDXF], I16)
            nc.sync.dma_start(gi[:], gidx.ap()[:, ck * IDXF:(ck + 1) * IDXF])
            grows = gpool.tile([P, SLOTS, tw], BF16)
            nc.gpsimd.dma_gather(grows[:], htab.ap()[r0:r1, :], gi[:],
                                 num_idxs=CHUNK, num_idxs_reg=CHUNK,
                                 elem_size=tw, single_packet=False)
            alt = apool.tile([P, SLOTS, n_heads], F32)
            nc.sync.dma_start(
                alt[:],
                al.ap()[:, ck * SLOTS * n_heads:(ck + 1) * SLOTS * n_heads]
                .rearrange("p (s h) -> p s h", h=n_heads))
            dlt = apool.tile([P, SLOTS, 1], BF16)
            nc.sync.dma_start(dlt[:, :, 0],
                              dstloc.ap()[:, ck * SLOTS:(ck + 1) * SLOTS])
            # ex = exp(max(al, 0.2*al))  [P, SLOTS, n_heads] bf16
            t1 = epool.tile([P, SLOTS, n_heads], F32)
            nc.vector.tensor_scalar_mul(t1[:], alt[:], 0.2)
            nc.vector.tensor_max(t1[:], t1[:], alt[:])
            ex = epool.tile([P, SLOTS, n_heads], BF16)
            nc.scalar.activation(ex[:], t1[:], AF.Exp)
            # Sw[e, d] = (iota == dstloc)  [P, SLOTS, P] bf16
            sw = spool.tile([P, SLOTS, P], BF16)
            a1, a2 = bass.broadcast_tensor_aps(iota_sb[:], dlt[:])
            nc.vector.tensor_tensor(sw[:], a1, a2, ALU.is_equal)
            # rhs = [ex_h * h_h | ex]  [P, SLOTS, rw] bf16
            rhs = rpool.tile([P, SLOTS, rw], BF16)
            for h in range(n_heads):
                b1, b2 = bass.broadcast_tensor_aps(
                    grows[:, :, h * hd:(h + 1) * hd], ex[:, :, h:h + 1])
                nc.vector.tensor_mul(rhs[:, :, h * hd:(h + 1) * hd], b1, b2)
            nc.vector.tensor_copy(rhs[:, :, hc:hc + n_heads], ex[:])
            # per-tile scatter matmuls
            for s in range(SLOTS):
                td = chunk_tiles[ck][s]
                if td is None:
                    # dummy tile: rhs is all zero (ex==0); skip only if no
                    # run is open; otherwise accumulate zeros to keep PE hot
                    continue
                blk, st, sp = td
                if st:
                    close_run()
                    pst = mpool.tile([P, rw], F32)
                    cur_ps = (pst, blk)
                else:
                    pst, _ = cur_ps
                nc.tensor.matmul(pst[:], sw[:, s, :], rhs[:, s, :],
                                 start=st, stop=sp)
        close_run()

        # phase 3: finalize
        fpool = ctx.enter_context(tc.tile_pool(name="fp", bufs=3))
        for b in range(n_blocks):
            rec = fpool.tile([P, n_heads], F32)
            nc.vector.tensor_scalar_add(rec[:], acc_sb[:, b, hc:hc + n_heads],
                                        1e-30)
            nc.vector.reciprocal(rec[:], rec[:])
            outt = fpool.tile([P, hc], F32)
            for h in range(n_heads):
                c1, c2 = bass.broadcast_tensor_aps(
                    acc_sb[:, b, h * hd:(h + 1) * hd], rec[:, h:h + 1])
                nc.vector.tensor_mul(outt[:, h * hd:(h + 1) * hd], c1, c2)
            nc.vector.tensor_add(outt[:], outt[:], bias_sb[:])
            if apply_elu:
                neg = fpool.tile([P, hc], F32)
                nc.vector.tensor_scalar_min(neg[:], outt[:], 0.0)
                enx = fpool.tile([P, hc], F32)
                nc.scalar.activation(enx[:], neg[:], AF.Exp)
                nc.vector.tensor_scalar_add(enx[:], enx[:], -1.0)
                nc.vector.tensor_scalar_max(outt[:], outt[:], 0.0)
                nc.vector.tensor_add(outt[:], outt[:], enx[:])
            nc.sync.dma_start(out.ap()[b * P:(b + 1) * P, :], outt[:])

    nc.compile()
    return nc


def _wrap_edge_scalars(v, n_chunks, width=1, dtype=np.float32):
    """[n_chunks*CHUNK(, width)] -> [P, n_chunks*SLOTS*width] wrap order."""
    v = v.reshape(n_chunks * CHUNK, width)
    outs = []
    for k in range(n_chunks):
        c = v[k * CHUNK:(k + 1) * CHUNK]           # [CHUNK, width]
        outs.append(c.reshape(SLOTS, P, width).transpose(1, 0, 2)
                    .reshape(P, SLOTS * width))
    return np.ascontiguousarray(np.concatenate(outs, axis=1), dtype=dtype)


def _wrap_idx(v, n_chunks):
    outs = []
    for k in range(n_chunks):
        c = v[k * CHUNK:(k + 1) * CHUNK]
        outs.append(np.tile(c.reshape(IDXF, 16).T, (8, 1)))
    return np.ascontiguousarray(np.concatenate(outs, axis=1), dtype=np.int16)


def prep_layer_inputs(n_nodes, x, W_np, bias_np, al_np, src, dst,
                      n_heads, hc, tw, n_shards=N_CORES):
    """Build per-core in_maps + compile-time tile structure."""
    n_nodes_pad = _ceil_to(n_nodes, P)
    shard_size = n_nodes // n_shards
    assert shard_size * n_shards == n_nodes
    n_blocks = _ceil_to(shard_size, P) // P
    out_rows = n_blocks * P
    n_groups = (n_nodes_pad + SRC_CHUNK - 1) // SRC_CHUNK

    xT = np.zeros((P, n_nodes_pad), np.float32)
    xT[:, :n_nodes] = x.T
    xT = xT.astype(np.dtype("bfloat16"))
    biasrep = np.tile(np.asarray(bias_np, np.float32)[None, :], (P, 1))
    Wf = np.ascontiguousarray(W_np).astype(np.dtype("bfloat16"))
    iotaT = np.tile(np.arange(P, dtype=np.float32)[None, :], (P, 1)).astype(
        np.dtype("bfloat16"))

    shard_of = dst // shard_size
    group_of = src // SRC_CHUNK
    block_of = (dst % shard_size) // P

    # per (core, group, block) edge lists
    per = {}
    for c in range(n_shards):
        m = shard_of == c
        s_c, d_c, al_c, g_c, b_c = (src[m], dst[m] % shard_size, al_np[m],
                                    group_of[m], block_of[m])
        order = np.lexsort((b_c,))          # stable by block
        for q in range(n_groups):
            mq = g_c == q
            sq, dq, alq, bq = s_c[mq], d_c[mq], al_c[mq], b_c[mq]
            o = np.argsort(bq, kind="stable")
            per[(c, q)] = (sq[o] - q * SRC_CHUNK, dq[o], alq[o], bq[o])

    # tiles per (group, block): max over cores
    tiles_gb = np.zeros((n_groups, n_blocks), np.int64)
    for c in range(n_shards):
        for q in range(n_groups):
            bq = per[(c, q)][3]
            cnt = np.bincount(bq, minlength=n_blocks)
            tiles_gb[q] = np.maximum(tiles_gb[q], -(-cnt // P))

    # compile-time chunk/tile structure (same for all cores)
    tile_desc = []     # (group, block, start, stop) per tile
    for q in range(n_groups):
        for b in range(n_blocks):
            t = int(tiles_gb[q, b])
            for i in range(t):
                tile_desc.append((q, b, i == 0, i == t - 1))
    # pad each group's tile list to chunk multiples with dummy tiles
    chunk_tiles, chunk_group = [], []
    cur, cur_q = [], None
    for q in range(n_groups):
        gts = [td for td in tile_desc if td[0] == q]
        npad = (-len(gts)) % SLOTS
        gts = gts + [None] * npad
        for i in range(0, len(gts), SLOTS):
            chunk_group.append(q)
            chunk_tiles.append([
                (td[1], td[2], td[3]) if td is not None else None
                for td in gts[i:i + SLOTS]])
    n_chunks = len(chunk_tiles)

    # per-core edge arrays following the tile structure
    in_maps = []
    for c in range(n_shards):
        gidx_c = np.zeros(n_chunks * CHUNK, np.int64)
        dl_c = np.zeros(n_chunks * CHUNK, np.float32)
        al_c = np.full((n_chunks * CHUNK, n_heads), -1e30, np.float32)
        # cursor into per[(c,q)] grouped by block
        for q in range(n_groups):
            sq, dq, alq, bq = per[(c, q)]
            boundaries = np.searchsorted(bq, np.arange(n_blocks + 1))
            # position of (q, b, i)-th tile in the global tile stream:
            pos = 0
            tpos = {}
            for (qq, b, st, sp) in tile_desc:
                if qq == q:
                    tpos.setdefault((q, b), pos)
                pos += 1 if qq == q else 0
            # map tiles to chunk slots
            # global slot index of tile j of group q:
            # account for chunk padding: group q's tiles start at the first
            # chunk with group q
            first_chunk = chunk_group.index(q)
            for b in range(n_blocks):
                e0, e1 = boundaries[b], boundaries[b + 1]
                t0 = tpos.get((q, b))
                if t0 is None:
                    continue
                for j in range(e1 - e0):
                    tj = t0 + j // P
                    slot = first_chunk * CHUNK + tj * P + (j % P)
                    gidx_c[slot] = sq[e0 + j]
                    dl_c[slot] = dq[e0 + j] - b * P
                    al_c[slot] = alq[e0 + j]
        im = {
            "xT": xT,
            "W": Wf,
            "biasrep": biasrep,
            "iotaT": iotaT,
            "gidx": _wrap_idx(gidx_c, n_chunks),
            "dstloc": _wrap_edge_scalars(dl_c, n_chunks,
                                         dtype=np.dtype("bfloat16")),
            "al": _wrap_edge_scalars(al_c, n_chunks, width=n_heads),
        }
        in_maps.append(im)

    cfg_part = dict(n_nodes_pad=n_nodes_pad, hc=hc, tw=tw, n_heads=n_heads,
                    head_dim=hc // n_heads, n_blocks=n_blocks,
                    out_rows=out_rows, chunk_tiles=chunk_tiles,
                    chunk_group=chunk_group)
    return in_maps, cfg_part, shard_size


def host_logits(x, W_np, We_np, a_s, a_d, a_e, src, dst, eattr, n_heads, hd):
    h = (x @ W_np).reshape(x.shape[0], n_heads, hd)
    asn = (h * a_s).sum(-1)
    adn = (h * a_d).sum(-1)
    ce = (We_np.reshape(n_heads, hd) * a_e[0]).sum(-1)
    return (asn[src] + adn[dst] + eattr[:, 0:1] * ce[None, :]).astype(np.float32)


def add_self_loops_np(src, dst, ew, n):
    deg = np.bincount(dst, minlength=n).astype(np.float32)
    sw = np.bincount(dst, weights=ew[:, 0], minlength=n).astype(np.float32)
    loop = sw / np.maximum(deg, 1.0)
    ar = np.arange(n, dtype=src.dtype)
    return (np.concatenate([src, ar]), np.concatenate([dst, ar]),
            np.concatenate([ew, loop[:, None].astype(np.float32)], axis=0))


def run_layer(x_in, W_np, bias_np, al_np, src, dst, n_heads, hc, tw,
              apply_elu, n_nodes):
    global LAST_EXEC_NS
    in_maps, cfg_part, shard_size = prep_layer_inputs(
        n_nodes, x_in, W_np, bias_np, al_np, src, dst, n_heads, hc, tw)
    cfg = dict(cfg_part, apply_elu=apply_elu)
    nc = build_layer_program(cfg)
    res = bass_utils.run_bass_kernel_spmd(nc, in_maps,
                                          core_ids=list(range(N_CORES)))
    outs = [res.results[c]["out"][:shard_size] for c in range(N_CORES)]
    return np.concatenate(outs, axis=0)


def kernel(**inputs):
    x = np.asarray(inputs["x"], np.float32)
    ei = np.asarray(inputs["edge_index"])
    ew = np.asarray(inputs["edge_weight"], np.float32)
    W1 = np.asarray(inputs["W1"], np.float32)
    We1 = np.asarray(inputs["We1"], np.float32)
    as1 = np.asarray(inputs["as1"], np.float32)
    ad1 = np.asarray(inputs["ad1"], np.float32)
    ae1 = np.asarray(inputs["ae1"], np.float32)
    b1 = np.asarray(inputs["b1"], np.float32)
    W2 = np.asarray(inputs["W2"], np.float32)
    We2 = np.asarray(inputs["We2"], np.float32)
    as2 = np.asarray(inputs["as2"], np.float32)
    ad2 = np.asarray(inputs["ad2"], np.float32)
    ae2 = np.asarray(inputs["ae2"], np.float32)
    b2 = np.asarray(inputs["b2"], np.float32)

    n = x.shape[0]
    src, dst, ea = add_self_loops_np(np.asarray(ei[0], np.int64),
                                     np.asarray(ei[1], np.int64), ew, n)

    al1 = host_logits(x, W1, We1, as1, ad1, ae1, src, dst, ea, H1, HID)
    h1 = run_layer(x, W1, b1, al1, src, dst, H1, H1 * HID, 128, True, n)

    al2 = host_logits(h1, W2, We2, as2, ad2, ae2, src, dst, ea, H2, OUT_DIM)
    out = run_layer(h1, W2, b2, al2, src, dst, H2, H2 * OUT_DIM, 128, False, n)
    return out
